# revision 42
# baseline (speedup 1.0000x reference)
"""GQA (H=32, KV=8, D=128, T=2048, hid=4096) causal attention + RoPE,
tensor-parallel over heads across 8 NeuronCores.

Sharding: core i owns kv-head i and query heads 4i..4i+3.

Pipeline (PE program order):
    qkv(c0..c3) | attn(q0) AG0 | attn(q1) AG1 | attn(q2) AG2 oproj(q0)
    | attn(q3) AG3 oproj(q1) | oproj(q2) oproj(q3)
so every AllGather flies under >=30us of compute and the PE stream never
waits on a collective.

Key differences vs the naive phase-serial version:
  - All DRAM inputs are HOST-PACKED into the exact SBUF layout, so every
    input DMA is one fully-contiguous run per partition (the unpacked
    rearranged gathers cost ~50us of PE idle at startup).
  - Softmax denominator via DVE accumulation of the exp tiles (fp32
    in-place adds, final bf16 copy) + ONE ones-matmul per (head,qchunk),
    replacing a per-tile PE ones-matmul (-65K PE cycles/core).
  - Attention is software-pipelined: S(i+2) issues before PV(i) so the
    Act-engine exp latency hides behind matmul streams; per-head
    normalization (bcast/recip/scale) is deferred into the next head.
  - Diagonal tiles are column-trimmed for ALL q-chunks (PSUM has_written
    semantics make partial-width accumulation safe in any order).
  - All matmul operands bf16 (fp8 was measured to break the 2e-2
    tolerance: attention here is peaked, logits up to ~8, so quantization
    noise does not average out), PSUM accumulation fp32.
Host concatenates the 8 column slices of o_proj output.
"""

import math
import numpy as np
import ml_dtypes

import concourse.bass as bass
import concourse.mybir as mybir
import concourse.tile as tile
from concourse import bacc
from concourse.bass_utils import run_bass_kernel_spmd

T = 2048
HID = 4096
H = 32
KV = 8
D = 128
NC = 8
HQ = H // NC          # 4 query heads per core
DQ = HQ * D           # 512
KT = HID // 128       # 32 contraction tiles
TC = T // 512         # 4 t-chunks
ROPE_BASE = 10000.0

MM_DT = mybir.dt.bfloat16
R32 = mybir.dt.float32r
F32 = mybir.dt.float32
EXP = mybir.ActivationFunctionType.Exp
MUL = mybir.AluOpType.mult
ADD = mybir.AluOpType.add

_BUILD_CACHE = {}
RUN_KWARGS = {}  # test harness hook (e.g. {"trace": True})


def _build_nc():
    nc = bacc.Bacc(None, target_bir_lowering=False, num_devices=NC)

    # host-packed inputs: every DMA below is contiguous per partition
    xP = nc.declare_dram_parameter("xP", [128, TC * KT * 512], MM_DT, isOutput=False)
    wq = nc.declare_dram_parameter("wq", [128, KT * DQ], MM_DT, isOutput=False)
    wk = nc.declare_dram_parameter("wk", [128, KT * D], MM_DT, isOutput=False)
    wv = nc.declare_dram_parameter("wv", [128, KT * D], MM_DT, isOutput=False)
    wo = nc.declare_dram_parameter("wo", [128, KT * DQ], MM_DT, isOutput=False)
    cosT = nc.declare_dram_parameter("cosT", [D, T], MM_DT, isOutput=False)
    sinT = nc.declare_dram_parameter("sinT", [D, T], MM_DT, isOutput=False)  # sign-folded
    masks = nc.declare_dram_parameter("masks", [128, 128], MM_DT, isOutput=False)
    ones = nc.declare_dram_parameter("ones", [128, 1], MM_DT, isOutput=False)
    onesr = nc.declare_dram_parameter("onesr", [1, 128], R32, isOutput=False)
    ident = nc.declare_dram_parameter("ident", [128, 128], F32, isOutput=False)
    out = nc.declare_dram_parameter("out", [T, DQ], F32, isOutput=True)

    # per-chunk attention output: [4 heads x 128 d, 512 t] -> gathered
    # [8 cores x 512, 512] with rows in original (core, head, d) order
    attT_loc = [nc.dram_tensor(f"attT_loc{c}", [DQ, 512], MM_DT)
                for c in range(TC)]
    attT_full = [nc.dram_tensor(f"attT_full{c}", [HID, 512], MM_DT,
                                addr_space="Shared") for c in range(TC)]

    inv_sqrt_d = 1.0 / math.sqrt(D)

    with tile.TileContext(nc) as tc:
        with (
            tc.tile_pool(name="persist", bufs=1) as pp,
            tc.tile_pool(name="mm", bufs=1, space="PSUM") as mm,
            tc.tile_pool(name="denp", bufs=2, space="PSUM") as denp,
            tc.tile_pool(name="xrhs", bufs=7) as xp,
            tc.tile_pool(name="ropetmp", bufs=1) as rp,
            tc.tile_pool(name="attn", bufs=4) as ap,
            tc.tile_pool(name="accp", bufs=2) as accp,
            tc.tile_pool(name="attout", bufs=2) as aop,
            tc.tile_pool(name="ostrip", bufs=3) as osp,
            tc.tile_pool(name="oout", bufs=2) as oop,
        ):
            # ---- persistent SBUF ----
            qt_sb = [[pp.tile([128, 512], MM_DT, tag=f"qt{h}_{c}",
                              name=f"qt{h}_{c}") for c in range(TC)]
                     for h in range(HQ)]
            kt_sb = [pp.tile([128, 512], MM_DT, tag=f"kt_{c}", name=f"kt_{c}")
                     for c in range(TC)]
            vt_sb = [pp.tile([128, 512], F32, tag=f"vt_{c}", name=f"vt_{c}")
                     for c in range(TC)]
            vn_sb = [pp.tile([128, 512], MM_DT, tag=f"vn_{c}", name=f"vn_{c}")
                     for c in range(TC)]
            cos_sb = pp.tile([128, T], MM_DT, tag="cos")
            sin_sb = pp.tile([128, T], MM_DT, tag="sin")
            msk_sb = pp.tile([128, 128], MM_DT, tag="msk")
            ones_sb = pp.tile([128, 1], MM_DT, tag="ones")
            onesr_sb = pp.tile([1, 128], R32, tag="onesr")
            id_sb = pp.tile([128, 128], F32, tag="ident")
            wq_sb = pp.tile([128, KT * DQ], MM_DT, tag="wq")
            wk_sb = pp.tile([128, KT * D], MM_DT, tag="wk")
            wv_sb = pp.tile([128, KT * D], MM_DT, tag="wv")
            wo_sb = pp.tile([128, KT * DQ], MM_DT, tag="wo")

            # mm-pool tag plan (all [128,512] F32, 6 banks):
            #   qkv:        pq0-3 -> A B C D, pk -> E, pv -> F
            #   V transp:   F
            #   attention:  s_ps cycles A B C, bc_ps D, o_ps alternates E F
            #   o_proj:     accumulators alternate A B
            def mmt(tag):
                return mm.tile([128, 512], F32, tag=tag, name=f"mm_{tag}")

            xt_pending = {}

            def issue_xt(tcn, ka):
                t = xp.tile([128, 4 * 512], MM_DT, tag="xt", name="xt")
                off = (tcn * 8 + ka) * 2048
                nc.sync.dma_start(t[:, :], xP[:, off:off + 2048])
                xt_pending[(tcn, ka)] = t

            def get_xt(tcn, ka):
                if (tcn, ka) not in xt_pending:
                    issue_xt(tcn, ka)
                return xt_pending.pop((tcn, ka))

            # ---- input DMAs, ordered for earliest PE start ----
            # sync queue: wk then x tiles; scalar queue: wv then wq (both
            # HW DGE); gpsimd (slow SW queue): small consts only
            nc.sync.dma_start(wk_sb[:, :], wk[:, :])
            nc.scalar.dma_start(wv_sb[:, :], wv[:, :])
            issue_xt(0, 0)
            issue_xt(0, 1)
            nc.gpsimd.dma_start(cos_sb[:, :], cosT[:, :])
            nc.gpsimd.dma_start(sin_sb[:, :], sinT[:, :])
            nc.gpsimd.dma_start(id_sb[:, :], ident[:, :])
            nc.gpsimd.dma_start(ones_sb[:, :], ones[:, :])
            nc.gpsimd.dma_start(onesr_sb[:, :], onesr[:, :])
            nc.gpsimd.dma_start(msk_sb[:, :], masks[:, :])
            for ka in range(2, 8):
                issue_xt(0, ka)
            QW = KT * DQ // 4
            for s in range(4):
                nc.scalar.dma_start(wq_sb[:, s * QW:(s + 1) * QW],
                                    wq[:, s * QW:(s + 1) * QW])

            # phase 1: all qkv chunks
            for tcn in range(TC):
                _qkv_chunk(nc, tcn, mmt, get_xt, rp, wq_sb, wk_sb, wv_sb,
                           qt_sb, kt_sb, vt_sb, vn_sb, cos_sb, sin_sb, id_sb)
                if tcn + 1 < TC:
                    for ka in range(4):
                        issue_xt(tcn + 1, ka)
            # wo load rides under the attention phase: the Act engine only
            # reaches this trigger after the qkv epilogue copies, so the 4MB
            # burst cannot starve the xt stream
            nc.scalar.dma_start(wo_sb[:, :], wo[:, :])

            # PE warmers: [1,512] matmuls that bridge the chunk-3 RoPE
            # epilogue drain (Act/DVE ~4.5us) so the HAM clock gate never
            # sees a >3.4us PE-idle window and attention starts at full
            # clock instead of K=4/8.  Anchored on kt_sb[3] (the FIRST
            # epilogue output) -- with no data dependency the Tile
            # scheduler hoists them to the program start, where they stall
            # on the slow const queue instead of bridging the boundary.
            for _w in range(14):
                warm = denp.tile([1, 512], F32, tag="den", name="warm")
                nc.tensor.matmul(warm[:, :], ones_sb[:, :],
                                 vn_sb[TC - 1][:, 0:512],
                                 start=True, stop=True,
                                 skip_group_check=True)

            # phase 2+3 interleaved: attention q-chunks with o_proj chunks
            # riding two chunks behind, so each AllGather hides under the
            # next ~30us+ of compute
            pending_fin = []

            def flush_one():
                if pending_fin:
                    pending_fin.pop(0)()

            def flush_fin():
                while pending_fin:
                    pending_fin.pop(0)()

            for qc in range(TC):
                for hp in (0, 2):
                    fins = _attn_pair(nc, hp, qc, mmt, denp, ap, accp, aop,
                                      qt_sb, kt_sb, vn_sb, msk_sb, ones_sb,
                                      onesr_sb, attT_loc, inv_sqrt_d,
                                      inject=flush_one)
                    pending_fin.extend(fins)
                flush_fin()  # attT_loc[qc] writes must precede the AG
                nc.gpsimd.collective_compute(
                    "AllGather",
                    mybir.AluOpType.bypass,
                    replica_groups=[list(range(NC))],
                    ins=[attT_loc[qc][:, :]],
                    outs=[attT_full[qc][:, :]],
                )
            # all o_proj after all attention: interleaving oproj between
            # attention chunks only delays attn3 (PE is serial) and thereby
            # AG3; sequential order lets every AG finish well before its
            # strips are needed (AG0 is done ~100us before oproj0 starts)
            for qc in range(TC):
                _oproj_chunk(nc, qc, mmt, osp, oop, wo_sb, attT_full, out)

    nc.compile()
    return nc


def _qkv_chunk(nc, tcn, mmt, get_xt, rp, wq_sb, wk_sb, wv_sb,
               qt_sb, kt_sb, vt_sb, vn_sb, cos_sb, sin_sb, id_sb):
    ts = tcn * 512
    pq = [mmt("ABCD"[h]) for h in range(HQ)]
    pk = mmt("E")
    pv = mmt("F")
    first_ka = 0
    if tcn == 0:
        # K/V projections for the first 8 k-blocks run while wq streams in
        first_ka = 2
        xts = [get_xt(0, ka) for ka in (0, 1)]
        # all K first, then all V: the wv DMA (2nd on the scalar queue)
        # lands ~8us after wk, so V work is deferred past the K burst
        for ki, xt4 in enumerate(xts):
            for j in range(4):
                k = 4 * ki + j
                xt = xt4[:, j * 512:(j + 1) * 512]
                nc.tensor.matmul(
                    pk[:, :], wk_sb[:, k * D:(k + 1) * D], xt,
                    start=(k == 0), stop=False, skip_group_check=True)
        for ki, xt4 in enumerate(xts):
            for j in range(4):
                k = 4 * ki + j
                xt = xt4[:, j * 512:(j + 1) * 512]
                nc.tensor.matmul(
                    pv[:, :], wv_sb[:, k * D:(k + 1) * D], xt,
                    start=(k == 0), stop=False, skip_group_check=True)
        for ki, xt4 in enumerate(xts):
            for j in range(4):
                k = 4 * ki + j
                xt = xt4[:, j * 512:(j + 1) * 512]
                for h in range(HQ):
                    nc.tensor.matmul(
                        pq[h][:, :],
                        wq_sb[:, k * DQ + h * 128: k * DQ + (h + 1) * 128],
                        xt,
                        start=(k == 0), stop=False, skip_group_check=True)
    for ka in range(first_ka, KT // 4):
        xt4 = get_xt(tcn, ka)
        for j in range(4):
            k = 4 * ka + j
            xt = xt4[:, j * 512:(j + 1) * 512]
            nc.tensor.matmul(
                pk[:, :], wk_sb[:, k * D:(k + 1) * D], xt,
                start=(k == 0), stop=(k == KT - 1), skip_group_check=True)
            nc.tensor.matmul(
                pv[:, :], wv_sb[:, k * D:(k + 1) * D], xt,
                start=(k == 0), stop=(k == KT - 1), skip_group_check=True)
            for h in range(HQ):
                nc.tensor.matmul(
                    pq[h][:, :],
                    wq_sb[:, k * DQ + h * 128: k * DQ + (h + 1) * 128],
                    xt,
                    start=(k == 0), stop=(k == KT - 1), skip_group_check=True)

    # V copy first: one Act op from PSUM, so the V-transpose -> vn chain
    # finishes ~0.9us after the last matmul and chunk 3's vn anchors the
    # PE warmers that much earlier
    nc.scalar.copy(vt_sb[tcn][:, :], pv[:, :])
    # RoPE epilogue: K first (unblocks attention S), then q heads
    for g in range(HQ + 1):
        src = pk if g == 0 else pq[g - 1]
        dst = kt_sb[tcn] if g == 0 else qt_sb[g - 1][tcn]
        qn_t = rp.tile([128, 512], F32, tag="qnat")
        nc.scalar.copy(qn_t[:, :], src[:, :])
        sh_t = rp.tile([128, 512], F32, tag="qshuf")
        nc.scalar.dma_start(sh_t[0:64, :], qn_t[64:128, :])
        nc.scalar.dma_start(sh_t[64:128, :], qn_t[0:64, :])
        qc_t = rp.tile([128, 512], F32, tag="qcos")
        nc.vector.tensor_tensor(
            qc_t[:, :], src[:, :], cos_sb[:, ts:ts + 512], op=MUL)
        ss_t = rp.tile([128, 512], F32, tag="qsin")
        nc.vector.tensor_tensor(
            ss_t[:, :], sh_t[:, :], sin_sb[:, ts:ts + 512], op=MUL)
        nc.vector.tensor_tensor(dst[:, :], qc_t[:, :], ss_t[:, :], op=ADD)

    # V transpose: 4x [128,128] into the F-tag PSUM bank, one copy out
    vp = mmt("F")
    for i in range(4):
        nc.tensor.transpose(
            vp[:, i * 128:(i + 1) * 128],
            vt_sb[tcn][:, i * 128:(i + 1) * 128], id_sb[:, :])
    nc.scalar.copy(vn_sb[tcn][:, :], vp[:, :])


def _attn_pair(nc, h0, qc, mmt, denp, ap, accp, aop, qt_sb, kt_sb, vn_sb,
               msk_sb, ones_sb, onesr_sb, attT_loc, inv_sqrt_d, inject=None):
    """Attention for heads (h0, h0+1) of q-chunk qc, software-pipelined
    with the two heads' tiles interleaved for 2x pipeline depth.

    S^T = K^T-stationary scores [k_tile, q], unnormalized softmax with the
    denominator accumulated on the DVE (in-place fp32), PV on the PE.

    Tile order per head: qc=0: diagonal m=0 full-width first (initializes
    the accumulator full width), then trimmed m=1..3.  qc>0: full-width
    off-diagonals ascending, trimmed m=1..3, then m=0 (cs=0) last.
    Units are (head, tile) pairs interleaved tile-major; PV(u) trails
    S(u)/exp(u) by 3 units so the Act exp latency is always covered.
    Returns two finalize closures (bcast/recip/scale/DMA) that the caller
    flushes during the NEXT pair so the PE never waits on the Act chain.
    """
    heads = (h0, h0 + 1)
    o_ps = [mmt("EF"[h % 2]) for h in heads]
    den_ps = [denp.tile([1, 512], F32, tag="den", name="den") for _ in heads]
    acc = [accp.tile([128, 512], F32, tag="acc", name="acc")
           for _ in heads]
    acc_bf = [accp.tile([128, 512], MM_DT, tag="accbf", name="accbf")
              for _ in heads]
    if qc == 0:
        order = [(0, 0)] + [(m, 128 * m) for m in (1, 2, 3)]
    else:
        order = [(kt, 0) for kt in range(4 * qc)]            # off-diag full
        order += [(4 * qc + m, 128 * m) for m in (1, 2, 3)]  # trimmed diag
        order += [(4 * qc, 0)]                               # m=0 last
    n = len(order)
    last = n - 1
    units = [(hl, i) for i in range(n) for hl in (0, 1)]
    nu = len(units)
    e_tiles = {}

    def emit_S(u):
        hl, i = units[u]
        kt, cs = order[i]
        s_ps = mmt("ABC"[u % 3])
        nc.tensor.matmul(
            s_ps[:, cs:512],
            kt_sb[kt // 4][:, (kt % 4) * 128:(kt % 4 + 1) * 128],
            qt_sb[heads[hl]][qc][:, cs:512],
            start=True, stop=True, skip_group_check=True)
        return s_ps

    def emit_exp(u, s_ps):
        hl, i = units[u]
        kt, cs = order[i]
        m = kt - 4 * qc
        e_t = ap.tile([128, 512], MM_DT, tag="et")
        if m >= 0:
            # diagonal: triangle block [cs, cs+128) masked + full tail
            e_raw = ap.tile([128, 128], F32, tag="erawn")
            nc.scalar.activation(e_raw[:, :], s_ps[:, cs:cs + 128], EXP,
                                 scale=inv_sqrt_d)
            nc.vector.tensor_tensor(
                e_t[:, cs:cs + 128], e_raw[:, :],
                msk_sb[:, 0:128], op=MUL)
            if cs + 128 < 512:
                nc.scalar.activation(
                    e_t[:, cs + 128:512], s_ps[:, cs + 128:512], EXP,
                    scale=inv_sqrt_d)
        else:
            nc.scalar.activation(e_t[:, :], s_ps[:, :], EXP,
                                 scale=inv_sqrt_d)
        # DVE denominator accumulation (in-place fp32)
        a = acc[hl]
        if i == 0:
            nc.vector.tensor_scalar_mul(a[:, cs:512], e_t[:, cs:512], 1.0)
        else:
            nc.vector.scalar_tensor_tensor(
                a[:, cs:512], e_t[:, cs:512], 1.0, a[:, cs:512],
                op0=MUL, op1=ADD)
        e_tiles[u] = e_t

    def emit_PV(u):
        hl, i = units[u]
        kt, cs = order[i]
        nc.tensor.matmul(
            o_ps[hl][:, cs:512],
            vn_sb[kt // 4][:, (kt % 4) * 128:(kt % 4 + 1) * 128],
            e_tiles.pop(u)[:, cs:512],
            start=(i == 0), stop=(i == last), skip_group_check=True)

    LOOK = 3
    for u in range(nu):
        s_ps = emit_S(u)
        emit_exp(u, s_ps)
        if u in (0, 2) and inject is not None:
            inject()  # previous pair's finalizes ride here, one at a time
        if u >= LOOK:
            emit_PV(u - LOOK)
    for u in range(max(0, nu - LOOK), nu):
        emit_PV(u)

    # denominator: bf16 copy of the fp32 accumulator, one ones-matmul
    for hl in (0, 1):
        nc.vector.tensor_scalar_mul(acc_bf[hl][:, :], acc[hl][:, :], 1.0)
        nc.tensor.matmul(den_ps[hl][:, :], ones_sb[:, :], acc_bf[hl][:, :],
                         start=True, stop=True, skip_group_check=True)
    den_sb = [aop.tile([1, 512], R32, tag="densb", name="densb")
              for _ in heads]
    for hl in (0, 1):
        nc.scalar.copy(den_sb[hl][:, :], den_ps[hl][:, :])

    def make_fin(hl):
        def finalize():
            # broadcast den via K=1 matmul, approx-reciprocal, scale, ship
            bc_ps = mmt("D")
            nc.tensor.matmul(
                bc_ps[:, :], onesr_sb[:, :], den_sb[hl][:, :],
                start=True, stop=True, skip_group_check=True)
            rcp_t = aop.tile([128, 512], F32, tag="rcp")
            nc.vector.reciprocal_approx_fast(rcp_t[:, :], bc_ps[:, :])
            at_t = aop.tile([128, 512], MM_DT, tag="attT")
            nc.vector.tensor_tensor(at_t[:, :], o_ps[hl][:, :], rcp_t[:, :],
                                    op=MUL)
            nc.gpsimd.dma_start(
                attT_loc[qc][heads[hl] * 128:(heads[hl] + 1) * 128, :],
                at_t[:, :])
        return finalize

    return [make_fin(0), make_fin(1)]


def _oproj_chunk(nc, qc, mmt, osp, oop, wo_sb, attT_full, out):
    att_r = attT_full[qc].rearrange("(a p) t -> p a t", p=128)
    strips = []
    for f in range(4):
        strip = osp.tile([128, KT * 128], MM_DT, tag="strip")
        nc.sync.dma_start(
            strip[:, :].rearrange("p (a f) -> p a f", a=KT),
            att_r[:, :, f * 128:(f + 1) * 128])
        strips.append(strip)
    for f in range(4):
        tt = qc * 4 + f
        strip = strips[f]
        o_ps = mmt("AB"[f % 2])
        for k2 in range(KT):
            nc.tensor.matmul(
                o_ps[:, :],
                strip[:, k2 * 128:(k2 + 1) * 128],
                wo_sb[:, k2 * DQ:(k2 + 1) * DQ],
                start=(k2 == 0), stop=(k2 == KT - 1), skip_group_check=True)
        ot = oop.tile([128, 512], F32, tag="ot")
        nc.scalar.copy(ot[:, :], o_ps[:, :])
        nc.scalar.dma_start(out[tt * 128:(tt + 1) * 128, :], ot[:, :])


def _host_consts():
    # rope tables, transposed + sign-folded
    inv = 1.0 / (ROPE_BASE ** (np.arange(0, D, 2, dtype=np.float32) / D))
    t = np.arange(T, dtype=np.float32)
    f = np.outer(t, inv)
    e = np.concatenate([f, f], axis=-1)
    cos = np.cos(e).astype(np.float32)
    sin = np.sin(e).astype(np.float32)
    sgn = np.where(np.arange(D) < D // 2, -1.0, 1.0).astype(np.float32)
    cosT = np.ascontiguousarray(cos.T)
    sinT = np.ascontiguousarray((sin * sgn).T)
    # local causal triangle for the trimmed diagonal blocks: keep iff f >= p
    p = np.arange(128)[:, None]
    fr = np.arange(128)[None, :]
    msk = (fr - p >= 0).astype(np.float32)
    ones = np.ones((128, 1), np.float32)
    onesr = np.ones((1, 128), np.float32)
    ident = np.eye(128, dtype=np.float32)
    return cosT, sinT, msk, ones, onesr, ident


def _pack_x(x):
    # xP[p, ((tcn*8+ka)*2048) + j*512 + t] = x[tcn*512+t, (4ka+j)*128+p]
    xr = x.reshape(T, HID).reshape(TC, 512, KT, 128)
    return np.ascontiguousarray(
        xr.transpose(3, 0, 2, 1).reshape(128, TC * KT * 512))


def _pack_w(w):
    # wP[p, k*F + c] = w[k*128+p, c]
    kt, f = w.shape[0] // 128, w.shape[1]
    return np.ascontiguousarray(
        w.reshape(kt, 128, f).transpose(1, 0, 2).reshape(128, kt * f))


def kernel(x, wq, wk, wv, wo, mask=None, **_ignored):
    BF16 = ml_dtypes.bfloat16
    x = np.asarray(x, dtype=np.float32)
    wq = np.asarray(wq, dtype=np.float32)
    wk = np.asarray(wk, dtype=np.float32)
    wv = np.asarray(wv, dtype=np.float32)
    wo = np.asarray(wo, dtype=np.float32)
    B = x.shape[0]
    xP = _pack_x(x).astype(BF16)
    cosT, sinT, msk, ones, onesr, ident = _host_consts()

    if "nc" not in _BUILD_CACHE:
        _BUILD_CACHE["nc"] = _build_nc()
    nc = _BUILD_CACHE["nc"]

    in_maps = []
    for i in range(NC):
        in_maps.append({
            "xP": xP,
            "wq": _pack_w(wq[:, i * DQ:(i + 1) * DQ]).astype(BF16),
            "wk": _pack_w(wk[:, i * D:(i + 1) * D]).astype(BF16),
            "wv": _pack_w(wv[:, i * D:(i + 1) * D]).astype(BF16),
            "wo": _pack_w(wo[:, i * DQ:(i + 1) * DQ]).astype(BF16),
            "cosT": cosT.astype(BF16), "sinT": sinT.astype(BF16),
            "masks": msk.astype(BF16),
            "ones": ones.astype(BF16), "onesr": onesr, "ident": ident,
        })

    res = run_bass_kernel_spmd(nc, in_maps, core_ids=list(range(NC)), **RUN_KWARGS)
    _BUILD_CACHE["last_res"] = res
    out = np.concatenate([res.results[i]["out"] for i in range(NC)], axis=1)
    return out.reshape(B, T, HID)


if __name__ == "__main__":
    rng = np.random.default_rng(0)
    s = 1.0 / math.sqrt(HID)
    x = rng.standard_normal((1, T, HID), dtype=np.float32)
    wq_ = rng.standard_normal((HID, H * D), dtype=np.float32) * s
    wk_ = rng.standard_normal((HID, KV * D), dtype=np.float32) * s
    wv_ = rng.standard_normal((HID, KV * D), dtype=np.float32) * s
    wo_ = rng.standard_normal((H * D, HID), dtype=np.float32) * s
    o = kernel(x, wq_, wk_, wv_, wo_, None)
    print("out", o.shape, o.dtype, float(np.abs(o).mean()))


# revision 43
# speedup vs baseline: 1.0878x; 1.0878x over previous
"""GQA (H=32, KV=8, D=128, T=2048, hid=4096) causal attention + RoPE,
tensor-parallel over heads across 8 NeuronCores.

Sharding: core i owns kv-head i and query heads 4i..4i+3.

Pipeline (PE program order):
    qkv(c0..c3) | attn(q0) AG0 | attn(q1) AG1 | attn(q2) AG2 oproj(q0)
    | attn(q3) AG3 oproj(q1) | oproj(q2) oproj(q3)
so every AllGather flies under >=30us of compute and the PE stream never
waits on a collective.

Key differences vs the naive phase-serial version:
  - All DRAM inputs are HOST-PACKED into the exact SBUF layout, so every
    input DMA is one fully-contiguous run per partition (the unpacked
    rearranged gathers cost ~50us of PE idle at startup).
  - Softmax denominator via DVE accumulation of the exp tiles (fp32
    in-place adds, final bf16 copy) + ONE ones-matmul per (head,qchunk),
    replacing a per-tile PE ones-matmul (-65K PE cycles/core).
  - Attention is software-pipelined: S(i+2) issues before PV(i) so the
    Act-engine exp latency hides behind matmul streams; per-head
    normalization (bcast/recip/scale) is deferred into the next head.
  - Diagonal tiles are column-trimmed for ALL q-chunks (PSUM has_written
    semantics make partial-width accumulation safe in any order).
  - All matmul operands bf16 (fp8 was measured to break the 2e-2
    tolerance: attention here is peaked, logits up to ~8, so quantization
    noise does not average out), PSUM accumulation fp32.
Host concatenates the 8 column slices of o_proj output.
"""

import math
import numpy as np
import ml_dtypes

import concourse.bass as bass
import concourse.mybir as mybir
import concourse.tile as tile
from concourse import bacc
from concourse.bass_utils import run_bass_kernel_spmd

T = 2048
HID = 4096
H = 32
KV = 8
D = 128
NC = 8
HQ = H // NC          # 4 query heads per core
DQ = HQ * D           # 512
KT = HID // 128       # 32 contraction tiles
TC = T // 512         # 4 t-chunks
ROPE_BASE = 10000.0

MM_DT = mybir.dt.bfloat16
R32 = mybir.dt.float32r
F32 = mybir.dt.float32
EXP = mybir.ActivationFunctionType.Exp
MUL = mybir.AluOpType.mult
ADD = mybir.AluOpType.add

_BUILD_CACHE = {}
RUN_KWARGS = {}  # test harness hook (e.g. {"trace": True})


def _build_nc():
    nc = bacc.Bacc(None, target_bir_lowering=False, num_devices=NC)

    # host-packed inputs: every DMA below is contiguous per partition
    xP = nc.declare_dram_parameter("xP", [128, TC * KT * 512], MM_DT, isOutput=False)
    wq = nc.declare_dram_parameter("wq", [128, KT * DQ], MM_DT, isOutput=False)
    wk = nc.declare_dram_parameter("wk", [128, KT * D], MM_DT, isOutput=False)
    wv = nc.declare_dram_parameter("wv", [128, KT * D], MM_DT, isOutput=False)
    wo = nc.declare_dram_parameter("wo", [128, KT * DQ], MM_DT, isOutput=False)
    cosT = nc.declare_dram_parameter("cosT", [D, T], MM_DT, isOutput=False)
    sinT = nc.declare_dram_parameter("sinT", [D, T], MM_DT, isOutput=False)  # sign-folded
    masks = nc.declare_dram_parameter("masks", [128, 4 * 512], MM_DT, isOutput=False)
    ones = nc.declare_dram_parameter("ones", [128, 1], MM_DT, isOutput=False)
    onesr = nc.declare_dram_parameter("onesr", [1, 128], R32, isOutput=False)
    ident = nc.declare_dram_parameter("ident", [128, 128], F32, isOutput=False)
    out = nc.declare_dram_parameter("out", [T, DQ], F32, isOutput=True)

    # per-chunk attention output: [4 heads x 128 d, 512 t] -> gathered
    # [8 cores x 512, 512] with rows in original (core, head, d) order
    attT_loc = [nc.dram_tensor(f"attT_loc{c}", [DQ, 512], MM_DT)
                for c in range(TC)]
    attT_full = [nc.dram_tensor(f"attT_full{c}", [HID, 512], MM_DT,
                                addr_space="Shared") for c in range(TC)]

    inv_sqrt_d = 1.0 / math.sqrt(D)

    with tile.TileContext(nc) as tc:
        with (
            tc.tile_pool(name="persist", bufs=1) as pp,
            tc.tile_pool(name="mm", bufs=1, space="PSUM") as mm,
            tc.tile_pool(name="denp", bufs=2, space="PSUM") as denp,
            tc.tile_pool(name="xrhs", bufs=6) as xp,
            tc.tile_pool(name="ropetmp", bufs=1) as rp,
            tc.tile_pool(name="attn", bufs=4) as ap,
            tc.tile_pool(name="accp", bufs=2) as accp,
            tc.tile_pool(name="attout", bufs=2) as aop,
            tc.tile_pool(name="ostrip", bufs=3) as osp,
            tc.tile_pool(name="oout", bufs=2) as oop,
        ):
            # ---- persistent SBUF ----
            qt_sb = [[pp.tile([128, 512], MM_DT, tag=f"qt{h}_{c}",
                              name=f"qt{h}_{c}") for c in range(TC)]
                     for h in range(HQ)]
            kt_sb = [pp.tile([128, 512], MM_DT, tag=f"kt_{c}", name=f"kt_{c}")
                     for c in range(TC)]
            vt_sb = [pp.tile([128, 512], F32, tag=f"vt_{c}", name=f"vt_{c}")
                     for c in range(TC)]
            vn_sb = [pp.tile([128, 512], MM_DT, tag=f"vn_{c}", name=f"vn_{c}")
                     for c in range(TC)]
            cos_sb = pp.tile([128, T], MM_DT, tag="cos")
            sin_sb = pp.tile([128, T], MM_DT, tag="sin")
            msk_sb = pp.tile([128, 2048], MM_DT, tag="msk")
            ones_sb = pp.tile([128, 1], MM_DT, tag="ones")
            onesr_sb = pp.tile([1, 128], R32, tag="onesr")
            id_sb = pp.tile([128, 128], F32, tag="ident")
            wq_sb = pp.tile([128, KT * DQ], MM_DT, tag="wq")
            wk_sb = pp.tile([128, KT * D], MM_DT, tag="wk")
            wv_sb = pp.tile([128, KT * D], MM_DT, tag="wv")
            wo_sb = pp.tile([128, KT * DQ], MM_DT, tag="wo")

            # mm-pool tag plan (all [128,512] F32, 6 banks):
            #   qkv:        pq0-3 -> A B C D, pk -> E, pv -> F
            #   V transp:   F
            #   attention:  s_ps cycles A B C, bc_ps D, o_ps alternates E F
            #   o_proj:     accumulators alternate A B
            def mmt(tag):
                return mm.tile([128, 512], F32, tag=tag, name=f"mm_{tag}")

            xt_pending = {}

            def issue_xt(tcn, ka):
                t = xp.tile([128, 4 * 512], MM_DT, tag="xt", name="xt")
                off = (tcn * 8 + ka) * 2048
                nc.sync.dma_start(t[:, :], xP[:, off:off + 2048])
                xt_pending[(tcn, ka)] = t

            def get_xt(tcn, ka):
                if (tcn, ka) not in xt_pending:
                    issue_xt(tcn, ka)
                return xt_pending.pop((tcn, ka))

            # ---- input DMAs, ordered for earliest PE start ----
            # sync queue: wk then x tiles; scalar queue: wv then wq (both
            # HW DGE); gpsimd (slow SW queue): small consts only
            nc.sync.dma_start(wk_sb[:, :], wk[:, :])
            nc.scalar.dma_start(wv_sb[:, :], wv[:, :])
            issue_xt(0, 0)
            issue_xt(0, 1)
            nc.gpsimd.dma_start(cos_sb[:, :], cosT[:, :])
            nc.gpsimd.dma_start(sin_sb[:, :], sinT[:, :])
            nc.gpsimd.dma_start(id_sb[:, :], ident[:, :])
            nc.gpsimd.dma_start(ones_sb[:, :], ones[:, :])
            nc.gpsimd.dma_start(onesr_sb[:, :], onesr[:, :])
            nc.gpsimd.dma_start(msk_sb[:, :], masks[:, :])
            for ka in range(2, 8):
                issue_xt(0, ka)
            QW = KT * DQ // 4
            for s in range(4):
                nc.scalar.dma_start(wq_sb[:, s * QW:(s + 1) * QW],
                                    wq[:, s * QW:(s + 1) * QW])

            # phase 1: all qkv chunks
            for tcn in range(TC):
                _qkv_chunk(nc, tcn, mmt, get_xt, rp, wq_sb, wk_sb, wv_sb,
                           qt_sb, kt_sb, vt_sb, vn_sb, cos_sb, sin_sb, id_sb)
                if tcn + 1 < TC:
                    for ka in range(4):
                        issue_xt(tcn + 1, ka)
            # wo load rides under the attention phase: the Act engine only
            # reaches this trigger after the qkv epilogue copies, so the 4MB
            # burst cannot starve the xt stream
            nc.scalar.dma_start(wo_sb[:, :], wo[:, :])

            # PE warmers: [1,512] matmuls that bridge the chunk-3 RoPE
            # epilogue drain (Act/DVE ~4.5us) so the HAM clock gate never
            # sees a >3.4us PE-idle window and attention starts at full
            # clock instead of K=4/8.  Anchored on kt_sb[3] (the FIRST
            # epilogue output) -- with no data dependency the Tile
            # scheduler hoists them to the program start, where they stall
            # on the slow const queue instead of bridging the boundary.
            for _w in range(14):
                warm = denp.tile([1, 512], F32, tag="den", name="warm")
                nc.tensor.matmul(warm[:, :], ones_sb[:, :],
                                 vn_sb[TC - 1][:, 0:512],
                                 start=True, stop=True,
                                 skip_group_check=True)

            # phase 2+3 interleaved: attention q-chunks with o_proj chunks
            # riding two chunks behind, so each AllGather hides under the
            # next ~30us+ of compute
            pending_fin = []

            def flush_one():
                if pending_fin:
                    pending_fin.pop(0)()

            def flush_fin():
                while pending_fin:
                    pending_fin.pop(0)()

            for qc in range(TC):
                for hp in (0, 2):
                    fins = _attn_pair(nc, hp, qc, mmt, denp, ap, accp, aop,
                                      qt_sb, kt_sb, vn_sb, msk_sb, ones_sb,
                                      onesr_sb, attT_loc, inv_sqrt_d,
                                      inject=flush_one)
                    pending_fin.extend(fins)
                flush_fin()  # attT_loc[qc] writes must precede the AG
                nc.gpsimd.collective_compute(
                    "AllGather",
                    mybir.AluOpType.bypass,
                    replica_groups=[list(range(NC))],
                    ins=[attT_loc[qc][:, :]],
                    outs=[attT_full[qc][:, :]],
                )
            # all o_proj after all attention: interleaving oproj between
            # attention chunks only delays attn3 (PE is serial) and thereby
            # AG3; sequential order lets every AG finish well before its
            # strips are needed (AG0 is done ~100us before oproj0 starts)
            for qc in range(TC):
                _oproj_chunk(nc, qc, mmt, osp, oop, wo_sb, attT_full, out)

    nc.compile()
    return nc


def _qkv_chunk(nc, tcn, mmt, get_xt, rp, wq_sb, wk_sb, wv_sb,
               qt_sb, kt_sb, vt_sb, vn_sb, cos_sb, sin_sb, id_sb):
    ts = tcn * 512
    pq = [mmt("ABCD"[h]) for h in range(HQ)]
    pk = mmt("E")
    pv = mmt("F")
    first_ka = 0
    if tcn == 0:
        # K/V projections for the first 8 k-blocks run while wq streams in
        first_ka = 2
        xts = [get_xt(0, ka) for ka in (0, 1)]
        # all K first, then all V: the wv DMA (2nd on the scalar queue)
        # lands ~8us after wk, so V work is deferred past the K burst
        for ki, xt4 in enumerate(xts):
            for j in range(4):
                k = 4 * ki + j
                xt = xt4[:, j * 512:(j + 1) * 512]
                nc.tensor.matmul(
                    pk[:, :], wk_sb[:, k * D:(k + 1) * D], xt,
                    start=(k == 0), stop=False, skip_group_check=True)
        for ki, xt4 in enumerate(xts):
            for j in range(4):
                k = 4 * ki + j
                xt = xt4[:, j * 512:(j + 1) * 512]
                nc.tensor.matmul(
                    pv[:, :], wv_sb[:, k * D:(k + 1) * D], xt,
                    start=(k == 0), stop=False, skip_group_check=True)
        for ki, xt4 in enumerate(xts):
            for j in range(4):
                k = 4 * ki + j
                xt = xt4[:, j * 512:(j + 1) * 512]
                for h in range(HQ):
                    nc.tensor.matmul(
                        pq[h][:, :],
                        wq_sb[:, k * DQ + h * 128: k * DQ + (h + 1) * 128],
                        xt,
                        start=(k == 0), stop=False, skip_group_check=True)
    for ka in range(first_ka, KT // 4):
        xt4 = get_xt(tcn, ka)
        for j in range(4):
            k = 4 * ka + j
            xt = xt4[:, j * 512:(j + 1) * 512]
            nc.tensor.matmul(
                pk[:, :], wk_sb[:, k * D:(k + 1) * D], xt,
                start=(k == 0), stop=(k == KT - 1), skip_group_check=True)
            nc.tensor.matmul(
                pv[:, :], wv_sb[:, k * D:(k + 1) * D], xt,
                start=(k == 0), stop=(k == KT - 1), skip_group_check=True)
            for h in range(HQ):
                nc.tensor.matmul(
                    pq[h][:, :],
                    wq_sb[:, k * DQ + h * 128: k * DQ + (h + 1) * 128],
                    xt,
                    start=(k == 0), stop=(k == KT - 1), skip_group_check=True)

    # V copy first: one Act op from PSUM, so the V-transpose -> vn chain
    # finishes ~0.9us after the last matmul and chunk 3's vn anchors the
    # PE warmers that much earlier
    nc.scalar.copy(vt_sb[tcn][:, :], pv[:, :])
    # RoPE epilogue: K first (unblocks attention S), then q heads
    for g in range(HQ + 1):
        src = pk if g == 0 else pq[g - 1]
        dst = kt_sb[tcn] if g == 0 else qt_sb[g - 1][tcn]
        qn_t = rp.tile([128, 512], F32, tag="qnat")
        nc.scalar.copy(qn_t[:, :], src[:, :])
        sh_t = rp.tile([128, 512], F32, tag="qshuf")
        nc.scalar.dma_start(sh_t[0:64, :], qn_t[64:128, :])
        nc.scalar.dma_start(sh_t[64:128, :], qn_t[0:64, :])
        qc_t = rp.tile([128, 512], F32, tag="qcos")
        nc.vector.tensor_tensor(
            qc_t[:, :], src[:, :], cos_sb[:, ts:ts + 512], op=MUL)
        ss_t = rp.tile([128, 512], F32, tag="qsin")
        nc.vector.tensor_tensor(
            ss_t[:, :], sh_t[:, :], sin_sb[:, ts:ts + 512], op=MUL)
        nc.vector.tensor_tensor(dst[:, :], qc_t[:, :], ss_t[:, :], op=ADD)

    # V transpose: 4x [128,128] into the F-tag PSUM bank, one copy out
    vp = mmt("F")
    for i in range(4):
        nc.tensor.transpose(
            vp[:, i * 128:(i + 1) * 128],
            vt_sb[tcn][:, i * 128:(i + 1) * 128], id_sb[:, :])
    nc.scalar.copy(vn_sb[tcn][:, :], vp[:, :])


def _attn_pair(nc, h0, qc, mmt, denp, ap, accp, aop, qt_sb, kt_sb, vn_sb,
               msk_sb, ones_sb, onesr_sb, attT_loc, inv_sqrt_d, inject=None):
    """Attention for heads (h0, h0+1) of q-chunk qc, software-pipelined
    with the two heads' tiles interleaved for 2x pipeline depth.

    S^T = K^T-stationary scores [k_tile, q], unnormalized softmax with the
    denominator accumulated on the DVE (in-place fp32), PV on the PE.

    Tile order per head: qc=0: diagonal m=0 full-width first (initializes
    the accumulator full width), then trimmed m=1..3.  qc>0: full-width
    off-diagonals ascending, trimmed m=1..3, then m=0 (cs=0) last.
    Units are (head, tile) pairs interleaved tile-major; PV(u) trails
    S(u)/exp(u) by 3 units so the Act exp latency is always covered.
    Returns two finalize closures (bcast/recip/scale/DMA) that the caller
    flushes during the NEXT pair so the PE never waits on the Act chain.
    """
    heads = (h0, h0 + 1)
    o_ps = [mmt("EF"[h % 2]) for h in heads]
    den_ps = [denp.tile([1, 512], F32, tag="den", name="den") for _ in heads]
    acc = [accp.tile([128, 512], F32, tag="acc", name="acc")
           for _ in heads]
    acc_bf = [accp.tile([128, 512], MM_DT, tag="accbf", name="accbf")
              for _ in heads]
    if qc == 0:
        order = [(0, 0)] + [(m, 128 * m) for m in (1, 2, 3)]
    else:
        order = [(kt, 0) for kt in range(4 * qc)]            # off-diag full
        order += [(4 * qc + m, 128 * m) for m in (1, 2, 3)]  # trimmed diag
        order += [(4 * qc, 0)]                               # m=0 last
    n = len(order)
    last = n - 1
    units = [(hl, i) for i in range(n) for hl in (0, 1)]
    nu = len(units)
    e_tiles = {}

    def emit_S(u):
        hl, i = units[u]
        kt, cs = order[i]
        s_ps = mmt("ABC"[u % 3])
        nc.tensor.matmul(
            s_ps[:, cs:512],
            kt_sb[kt // 4][:, (kt % 4) * 128:(kt % 4 + 1) * 128],
            qt_sb[heads[hl]][qc][:, cs:512],
            start=True, stop=True, skip_group_check=True)
        return s_ps

    def emit_exp(u, s_ps):
        hl, i = units[u]
        kt, cs = order[i]
        m = kt - 4 * qc
        e_t = ap.tile([128, 512], MM_DT, tag="et")
        if m >= 0:
            # diagonal: triangle block [cs, cs+128) masked + full tail
            e_raw = ap.tile([128, 128], F32, tag="erawn")
            nc.scalar.activation(e_raw[:, :], s_ps[:, cs:cs + 128], EXP,
                                 scale=inv_sqrt_d)
            nc.vector.tensor_tensor(
                e_t[:, cs:cs + 128], e_raw[:, :],
                msk_sb[:, 0:128], op=MUL)
            if cs + 128 < 512:
                nc.scalar.activation(
                    e_t[:, cs + 128:512], s_ps[:, cs + 128:512], EXP,
                    scale=inv_sqrt_d)
        else:
            nc.scalar.activation(e_t[:, :], s_ps[:, :], EXP,
                                 scale=inv_sqrt_d)
        # DVE denominator accumulation (in-place fp32)
        a = acc[hl]
        if i == 0:
            nc.vector.tensor_scalar_mul(a[:, cs:512], e_t[:, cs:512], 1.0)
        else:
            nc.vector.scalar_tensor_tensor(
                a[:, cs:512], e_t[:, cs:512], 1.0, a[:, cs:512],
                op0=MUL, op1=ADD)
        e_tiles[u] = e_t

    def emit_PV(u):
        hl, i = units[u]
        kt, cs = order[i]
        nc.tensor.matmul(
            o_ps[hl][:, cs:512],
            vn_sb[kt // 4][:, (kt % 4) * 128:(kt % 4 + 1) * 128],
            e_tiles.pop(u)[:, cs:512],
            start=(i == 0), stop=(i == last), skip_group_check=True)

    LOOK = 3
    for u in range(nu):
        s_ps = emit_S(u)
        emit_exp(u, s_ps)
        if u in (0, 2) and inject is not None:
            inject()  # previous pair's finalizes ride here, one at a time
        if u >= LOOK:
            emit_PV(u - LOOK)
    for u in range(max(0, nu - LOOK), nu):
        emit_PV(u)

    # denominator: bf16 copy of the fp32 accumulator, one ones-matmul
    for hl in (0, 1):
        nc.vector.tensor_scalar_mul(acc_bf[hl][:, :], acc[hl][:, :], 1.0)
        nc.tensor.matmul(den_ps[hl][:, :], ones_sb[:, :], acc_bf[hl][:, :],
                         start=True, stop=True, skip_group_check=True)
    den_sb = [aop.tile([1, 512], R32, tag="densb", name="densb")
              for _ in heads]
    for hl in (0, 1):
        nc.scalar.copy(den_sb[hl][:, :], den_ps[hl][:, :])

    def make_fin(hl):
        def finalize():
            # broadcast den via K=1 matmul, approx-reciprocal, scale, ship
            bc_ps = mmt("D")
            nc.tensor.matmul(
                bc_ps[:, :], onesr_sb[:, :], den_sb[hl][:, :],
                start=True, stop=True, skip_group_check=True)
            rcp_t = aop.tile([128, 512], F32, tag="rcp")
            nc.vector.reciprocal_approx_fast(rcp_t[:, :], bc_ps[:, :])
            at_t = aop.tile([128, 512], MM_DT, tag="attT")
            nc.vector.tensor_tensor(at_t[:, :], o_ps[hl][:, :], rcp_t[:, :],
                                    op=MUL)
            nc.gpsimd.dma_start(
                attT_loc[qc][heads[hl] * 128:(heads[hl] + 1) * 128, :],
                at_t[:, :])
        return finalize

    return [make_fin(0), make_fin(1)]


def _oproj_chunk(nc, qc, mmt, osp, oop, wo_sb, attT_full, out):
    att_r = attT_full[qc].rearrange("(a p) t -> p a t", p=128)
    strips = []
    for f in range(4):
        strip = osp.tile([128, KT * 128], MM_DT, tag="strip")
        nc.sync.dma_start(
            strip[:, :].rearrange("p (a f) -> p a f", a=KT),
            att_r[:, :, f * 128:(f + 1) * 128])
        strips.append(strip)
    for f in range(4):
        tt = qc * 4 + f
        strip = strips[f]
        o_ps = mmt("AB"[f % 2])
        for k2 in range(KT):
            nc.tensor.matmul(
                o_ps[:, :],
                strip[:, k2 * 128:(k2 + 1) * 128],
                wo_sb[:, k2 * DQ:(k2 + 1) * DQ],
                start=(k2 == 0), stop=(k2 == KT - 1), skip_group_check=True)
        ot = oop.tile([128, 512], F32, tag="ot")
        nc.scalar.copy(ot[:, :], o_ps[:, :])
        nc.scalar.dma_start(out[tt * 128:(tt + 1) * 128, :], ot[:, :])


def _host_consts():
    # rope tables, transposed + sign-folded
    inv = 1.0 / (ROPE_BASE ** (np.arange(0, D, 2, dtype=np.float32) / D))
    t = np.arange(T, dtype=np.float32)
    f = np.outer(t, inv)
    e = np.concatenate([f, f], axis=-1)
    cos = np.cos(e).astype(np.float32)
    sin = np.sin(e).astype(np.float32)
    sgn = np.where(np.arange(D) < D // 2, -1.0, 1.0).astype(np.float32)
    cosT = np.ascontiguousarray(cos.T)
    sinT = np.ascontiguousarray((sin * sgn).T)
    # causal 0/1 masks for the 4 diagonal kt-tile classes: keep iff f - p >= 128*m
    p = np.arange(128)[:, None]
    fr = np.arange(512)[None, :]
    msk = np.concatenate(
        [(fr - p >= 128 * m).astype(np.float32) for m in range(4)], axis=1)
    ones = np.ones((128, 1), np.float32)
    onesr = np.ones((1, 128), np.float32)
    ident = np.eye(128, dtype=np.float32)
    return cosT, sinT, msk, ones, onesr, ident


def _pack_x(x):
    # xP[p, ((tcn*8+ka)*2048) + j*512 + t] = x[tcn*512+t, (4ka+j)*128+p]
    xr = x.reshape(T, HID).reshape(TC, 512, KT, 128)
    return np.ascontiguousarray(
        xr.transpose(3, 0, 2, 1).reshape(128, TC * KT * 512))


def _pack_w(w):
    # wP[p, k*F + c] = w[k*128+p, c]
    kt, f = w.shape[0] // 128, w.shape[1]
    return np.ascontiguousarray(
        w.reshape(kt, 128, f).transpose(1, 0, 2).reshape(128, kt * f))


def kernel(x, wq, wk, wv, wo, mask=None, **_ignored):
    BF16 = ml_dtypes.bfloat16
    x = np.asarray(x, dtype=np.float32)
    wq = np.asarray(wq, dtype=np.float32)
    wk = np.asarray(wk, dtype=np.float32)
    wv = np.asarray(wv, dtype=np.float32)
    wo = np.asarray(wo, dtype=np.float32)
    B = x.shape[0]
    xP = _pack_x(x).astype(BF16)
    cosT, sinT, msk, ones, onesr, ident = _host_consts()

    if "nc" not in _BUILD_CACHE:
        _BUILD_CACHE["nc"] = _build_nc()
    nc = _BUILD_CACHE["nc"]

    in_maps = []
    for i in range(NC):
        in_maps.append({
            "xP": xP,
            "wq": _pack_w(wq[:, i * DQ:(i + 1) * DQ]).astype(BF16),
            "wk": _pack_w(wk[:, i * D:(i + 1) * D]).astype(BF16),
            "wv": _pack_w(wv[:, i * D:(i + 1) * D]).astype(BF16),
            "wo": _pack_w(wo[:, i * DQ:(i + 1) * DQ]).astype(BF16),
            "cosT": cosT.astype(BF16), "sinT": sinT.astype(BF16),
            "masks": msk.astype(BF16),
            "ones": ones.astype(BF16), "onesr": onesr, "ident": ident,
        })

    res = run_bass_kernel_spmd(nc, in_maps, core_ids=list(range(NC)), **RUN_KWARGS)
    _BUILD_CACHE["last_res"] = res
    out = np.concatenate([res.results[i]["out"] for i in range(NC)], axis=1)
    return out.reshape(B, T, HID)


if __name__ == "__main__":
    rng = np.random.default_rng(0)
    s = 1.0 / math.sqrt(HID)
    x = rng.standard_normal((1, T, HID), dtype=np.float32)
    wq_ = rng.standard_normal((HID, H * D), dtype=np.float32) * s
    wk_ = rng.standard_normal((HID, KV * D), dtype=np.float32) * s
    wv_ = rng.standard_normal((HID, KV * D), dtype=np.float32) * s
    wo_ = rng.standard_normal((H * D, HID), dtype=np.float32) * s
    o = kernel(x, wq_, wk_, wv_, wo_, None)
    print("out", o.shape, o.dtype, float(np.abs(o).mean()))
